# revision 1
# baseline (speedup 1.0000x reference)
"""Trainium2 Bass kernel v2 for DiffusionLoss (L1 noise loss + chamfer distance).

Contract: kernel(**inputs) takes the FULL [8, 16384, 3] f32 inputs, shards the
batch across 8 NeuronCores (1 batch element per core), and returns the full
scalar loss (shape () float32). Host sums the 8 per-core partial scalars.

This target pays a large (~40us) dispatch cost per STATIC instruction, while
For_i hardware-loop iterations run at near-hardware speed (~30us/back-edge).
v2 therefore restructures the baseline's fully unrolled ~1.3k-instruction
stream into a small static program (~100 instrs) + For_i band loops.

Math: distances are built as full squared distances via the ScalarEngine:
  sq_d[p, j] = Square(b_d[j] * (-1) + a_d[i])    (ACT, per-partition bias)
  D[p, j]    = sq_x + sq_y + sq_z                (DVE bf16 adds, 2x mode)
so no |a|^2 / |b|^2 tiles, no cancellation (bf16-safe: D is exact-relative),
no relu (D >= 0 by construction), and the ACT engine runs in parallel with
the DVE adds / min-reductions. Per band t (128 pred points i = 128t + p):
  rm[p, t]     = min_j D[p, j]                  (row mins; summed at end)
  colacc[p, j] = min(colacc[p, j], D[p, j])     (col mins over bands;
                                                 partition-reduced at end)
Targets j are processed in two halves of 8192 to fit SBUF; b coords are
bf16-quantized (slightly perturbed target points — exact geometry on the
perturbed cloud, fine at the 2e-2 tolerance).
"""

import numpy as np
from contextlib import ExitStack

import concourse.bacc as bacc
import concourse.bass as bass
import concourse.bass_isa as bass_isa
import concourse.mybir as mybir
import concourse.tile as tile
from concourse.bass_utils import run_bass_kernel_spmd

F32 = mybir.dt.float32
BF16 = mybir.dt.bfloat16
A = mybir.AluOpType
AX = mybir.AxisListType
AF = mybir.ActivationFunctionType
ds = bass.ds

B = 8
N = 16384
NCORES = 8
P = 128
BIG = 3.0e38
JH = 8192          # j-half width
U = 4              # bands per For_i iteration

NOISE_WEIGHT = 1.0
CHAMFER_WEIGHT = 0.1


def diffusion_loss_kernel(ctx, tc, out_ap, ins, n=N, variant="full",
                          u=U, pairred=False, stagger=False):
    do_act = variant not in ("noact", "static")
    do_dve = variant not in ("nodve", "static")
    do_loop = variant != "static"
    nc = tc.nc
    assert n % P == 0
    nt = n // P            # number of i-bands
    jh = min(JH, n)
    nh = n // jh           # number of j-halves
    npp = n // P
    wn = float(NOISE_WEIGHT / (B * n * 3))
    wc = float(CHAMFER_WEIGHT / (B * n))
    u = min(u, nt)

    persist = ctx.enter_context(tc.tile_pool(name="persist", bufs=1))

    # ---------------- noise L1 loss ----------------
    noiseacc = persist.tile([P, 1], F32)
    with tc.tile_pool(name="noise", bufs=1) as nprep:
        pn_nat = nprep.tile([P, 3 * npp], F32)
        an_nat = nprep.tile([P, 3 * npp], F32)
        nc.sync.dma_start(pn_nat[:], ins["pn"].rearrange("(p f) d -> p (f d)", p=P))
        nc.sync.dma_start(an_nat[:], ins["an"].rearrange("(p f) d -> p (f d)", p=P))
        nc.vector.tensor_sub(pn_nat[:], pn_nat[:], an_nat[:])
        nc.vector.tensor_reduce(
            noiseacc[:], pn_nat[:], axis=AX.X, op=A.add, apply_absolute_value=True
        )

    # ---------------- persistent tiles ----------------
    # acols[p, d, t] = pred coord d of point i = 128*t + p
    acols = persist.tile([P, 3, nt], F32)
    nc.sync.dma_start(acols[:], ins["pred"].rearrange("(t p) d -> p d t", p=P))
    # ACT bias APs cannot take register (loop-var) offsets, so the bias
    # scalars for each iteration's u bands are staged to a fixed address
    # by the otherwise-idle Pool engine, one copy per iteration.
    stage = persist.tile([P, 3, u], F32)
    rm = persist.tile([P, nt], F32)          # per-band row mins
    rtmp2 = persist.tile([P, 2], F32)
    colacc = persist.tile([P, nh, jh], BF16)  # running col mins (j = h*jh + jj)
    nc.vector.memset(colacc.rearrange("p a b -> p (a b)"), BIG)
    b3 = persist.tile([P, 3, jh], BF16)       # b coords for current half, bcast

    with tc.tile_pool(name="main", bufs=1) as main:
        Aq = main.tile([P, 2, jh], BF16)      # D accumulator, band-parity dbuf
        Bq = main.tile([P, 3, jh], BF16)      # sq_y (parity dbuf) / sq_z
        if not (do_act and do_dve):
            nc.vector.memset(rm[:], 0.0)
            nc.vector.memset(Aq.rearrange("p a b -> p (a b)"), 1.0)
            nc.vector.memset(Bq.rearrange("p a b -> p (a b)"), 1.0)

        for h in range(nh):
            jb = h * jh
            # ---- load + bf16-cast + broadcast this half's target coords ----
            with tc.tile_pool(name="bprep", bufs=1) as bprep:
                scr = bprep.tile([1, jh], F32)
                for c in range(3):
                    nc.sync.dma_start(
                        scr[:],
                        ins["targ"][jb : jb + jh, c : c + 1].rearrange("j o -> o j"),
                    )
                    nc.vector.tensor_copy(b3[0:1, c, :], scr[:])
                nc.gpsimd.partition_broadcast(
                    b3.rearrange("p a b -> p (a b)"),
                    b3[0:1, :, :].rearrange("p a b -> p (a b)"),
                    channels=P,
                )

            # ---- band loop ----
            if not do_loop:
                continue
            with tc.For_i(0, nt, u, staggered_reset=stagger) as t0:
                # one Pool staging copy for all u bands' bias scalars
                nc.gpsimd.tensor_copy(stage[:, :, :], acols[:, :, ds(t0, u)])
                for k in range(u // 2):
                    for pi in (0, 1):
                        uu = 2 * k + pi
                        if do_act:
                            nc.scalar.activation(
                                Aq[:, pi, :], b3[:, 0, :], AF.Square,
                                bias=stage[:, 0, uu : uu + 1], scale=-1.0,
                            )
                            nc.scalar.activation(
                                Bq[:, pi, :], b3[:, 1, :], AF.Square,
                                bias=stage[:, 1, uu : uu + 1], scale=-1.0,
                            )
                            nc.scalar.activation(
                                Bq[:, 2, :], b3[:, 2, :], AF.Square,
                                bias=stage[:, 2, uu : uu + 1], scale=-1.0,
                            )
                        if do_dve:
                            nc.vector.tensor_add(
                                Aq[:, pi, :], Aq[:, pi, :], Bq[:, pi, :]
                            )
                            nc.vector.tensor_add(
                                Aq[:, pi, :], Aq[:, pi, :], Bq[:, 2, :]
                            )
                            nc.vector.tensor_tensor(
                                out=colacc[:, h, :], in0=colacc[:, h, :],
                                in1=Aq[:, pi, :], op=A.min,
                            )
                            if not pairred:
                                if h == 0:
                                    nc.vector.tensor_reduce(
                                        rm[:, ds(t0 + uu, 1)], Aq[:, pi, :],
                                        axis=AX.X, op=A.min,
                                    )
                                else:
                                    nc.vector.tensor_reduce(
                                        rtmp2[:, 0:1], Aq[:, pi, :],
                                        axis=AX.X, op=A.min,
                                    )
                                    nc.vector.tensor_tensor(
                                        out=rm[:, ds(t0 + uu, 1)],
                                        in0=rm[:, ds(t0 + uu, 1)],
                                        in1=rtmp2[:, 0:1], op=A.min,
                                    )
                    if do_dve and pairred:
                        # one 2-band row-min reduce per pair
                        if h == 0:
                            nc.vector.tensor_reduce(
                                rm[:, ds(t0 + 2 * k, 2)], Aq[:, :, :],
                                axis=AX.X, op=A.min,
                            )
                        else:
                            nc.vector.tensor_reduce(
                                rtmp2[:], Aq[:, :, :], axis=AX.X, op=A.min
                            )
                            nc.vector.tensor_tensor(
                                out=rm[:, ds(t0 + 2 * k, 2)],
                                in0=rm[:, ds(t0 + 2 * k, 2)],
                                in1=rtmp2[:], op=A.min,
                            )

        # ---------------- epilogue ----------------
        # col mins: negate, partition-max, sum
        negc = Bq[:, 0:2, :].rearrange("p a b -> p (a b)")
        nc.vector.tensor_scalar_mul(
            negc, colacc.rearrange("p a b -> p (a b)"), -1.0
        )
        posm = Aq.rearrange("p a b -> p (a b)")
        nc.gpsimd.partition_all_reduce(
            posm, negc, channels=P, reduce_op=bass_isa.ReduceOp.max
        )
        csum = persist.tile([1, 1], F32)
        nc.vector.tensor_reduce(
            csum[:], Aq[0:1, :, :].rearrange("p a b -> p (a b)"),
            axis=AX.X, op=A.add,
        )

        # row mins: sum over bands
        racc = persist.tile([P, 1], F32)
        nc.vector.tensor_reduce(racc[:], rm[:], axis=AX.X, op=A.add)

        # combine: per-partition v = noise*wn + rowsum*wc, then partition-sum
        v1 = persist.tile([P, 1], F32)
        nc.vector.tensor_scalar_mul(v1[:], noiseacc[:], wn)
        v = persist.tile([P, 1], F32)
        nc.vector.scalar_tensor_tensor(
            out=v[:], in0=racc[:], scalar=wc, in1=v1[:], op0=A.mult, op1=A.add
        )
        vv = persist.tile([P, 1], F32)
        nc.gpsimd.partition_all_reduce(
            vv[:], v[:], channels=P, reduce_op=bass_isa.ReduceOp.add
        )
        # csum holds sum_j(-colmin_j): out = vv - wc*csum... (-wc)*csum + vv
        fs = persist.tile([1, 1], F32)
        nc.vector.scalar_tensor_tensor(
            out=fs[:], in0=csum[:], scalar=-wc, in1=vv[0:1, :],
            op0=A.mult, op1=A.add,
        )
        nc.sync.dma_start(out_ap, fs[:])


_CACHE = {}


def build_program(n=N, variant="full", u=U, pairred=False, stagger=False):
    key = (n, variant, u, pairred, stagger)
    if key not in _CACHE:
        nc = bacc.Bacc(
            "TRN2", target_bir_lowering=False, debug=False, enable_asserts=False
        )
        ins = {
            name: nc.dram_tensor(name, [n, 3], F32, kind="ExternalInput").ap()
            for name in ("pn", "an", "pred", "targ")
        }
        out_ap = nc.dram_tensor("out", [1, 1], F32, kind="ExternalOutput").ap()
        with tile.TileContext(nc) as tc:
            with ExitStack() as ctx:
                diffusion_loss_kernel(ctx, tc, out_ap, ins, n=n, variant=variant,
                                      u=u, pairred=pairred, stagger=stagger)
        nc.compile()
        _CACHE[key] = nc
    return _CACHE[key]


def run_cores(inputs, n=N, trace=False):
    """Run the SPMD program over the batch; returns (partials, results)."""
    nc = build_program(n=n)
    pn = np.ascontiguousarray(np.asarray(inputs["predicted_noise"], np.float32))
    an = np.ascontiguousarray(np.asarray(inputs["actual_noise"], np.float32))
    pred = np.ascontiguousarray(
        np.asarray(inputs["predicted_points_coarse"], np.float32)
    )
    targ = np.ascontiguousarray(
        np.asarray(inputs["target_points_coarse"], np.float32)
    )
    in_maps = [
        {"pn": pn[b], "an": an[b], "pred": pred[b], "targ": targ[b]}
        for b in range(pn.shape[0])
    ]
    res = run_bass_kernel_spmd(
        nc, in_maps, core_ids=list(range(len(in_maps))), trace=trace
    )
    partials = np.array(
        [res.results[b]["out"][0, 0] for b in range(len(in_maps))], np.float32
    )
    return partials, res


def kernel(predicted_noise, actual_noise, predicted_points_coarse,
           target_points_coarse):
    partials, _ = run_cores(
        {
            "predicted_noise": predicted_noise,
            "actual_noise": actual_noise,
            "predicted_points_coarse": predicted_points_coarse,
            "target_points_coarse": target_points_coarse,
        }
    )
    return np.array(np.sum(partials, dtype=np.float32), dtype=np.float32)



# revision 20
# speedup vs baseline: 33.2125x; 33.2125x over previous
"""Trainium2 Bass kernel v5 for DiffusionLoss (L1 noise loss + chamfer distance).

Contract: kernel(**inputs) takes the FULL [8, 16384, 3] f32 inputs, shards the
batch across 8 NeuronCores (1 batch element per core), and returns the full
scalar loss (shape () float32). Host sums the 8 per-core partial scalars.

Distance matrix on the TensorEngine via an augmented K=5 matmul:
  lhsT col i = [-2*a_x, -2*a_y, -2*a_z, |a|^2, 1]   (pred point i, bf16)
  rhs  col j = [ b_x,    b_y,    b_z,   1,  |b|^2]  (target point j, bf16)
one matmul emits the complete squared-distance tile
  D[i, j] = |a_i|^2 - 2 a.b + |b_j|^2
into PSUM (f32).  Per 128-point pred band, both chamfer reductions come out
of a SINGLE ScalarE pass over each PSUM tile:
  - ACT computes dt = exp(-K*D) -> SBUF bf16, and its accum_out sums each
    row -> softmin:  min_j D ~= -ln(sum_j exp(-K*D))/K   (bias ln(Neff)/K,
    ~2e-3 absolute; chamfer is only ~0.3% of the loss so this is ~2e-4 rel).
  - VectorE (2x bf16) max-folds dt into colacc[128, 16384]; since exp is
    monotone, max exp(-K*D) == exp(-K*min D), recovered exactly by the
    epilogue Ln.  colacc pre-set to TINY so Ln never sees 0 (outliers cap
    at -ln(TINY)/K ~ 0.29, a ~1e-4 rel effect).
Tail: GpSimd partition_all_reduce(max) + one-lane Ln + sum.

ScalarE is the bottleneck (~1.2 G elem/s/lane x 128): every D element must
cross PSUM->SBUF exactly once, and this design fuses BOTH reductions into
that one crossing.  Device time ~2.3 ms/core (sim), vs 15.9 ms baseline.

Runs are dispatched through a cached jax.jit(shard_map) callable -- building
it fresh each call (as bass_utils.run_bass_kernel_spmd does) re-traces and
re-loads the program at ~20-70us per static instruction through the axon
tunnel; caching removes that per-run cost entirely.

NOTE: tensor_tensor_reduce faults on real HW in this stack (verified by
micro-test; both SBUF and PSUM operand variants) -- do not reintroduce it.
"""

import numpy as np
from contextlib import ExitStack

import jax
from jax.sharding import Mesh, PartitionSpec
from jax.experimental.shard_map import shard_map

import concourse.bacc as bacc
import concourse.bass_isa as bass_isa
import concourse.mybir as mybir
import concourse.tile as tile
from concourse import bass2jax

F32 = mybir.dt.float32
BF16 = mybir.dt.bfloat16
A = mybir.AluOpType
AX = mybir.AxisListType
AF = mybir.ActivationFunctionType

B = 8
N = 16384
NCORES = 8
P = 128
FD = 512            # matmul moving free dim (one PSUM-bank f32)
G = 2048            # per-group j width (4 matmuls, 4 PSUM banks)
K_SOFT = 300.0      # softmin sharpness for the exp-domain row reduction
TINY = 2.0e-38      # bf16-representable floor for exp-domain col mins

NOISE_WEIGHT = 1.0
CHAMFER_WEIGHT = 0.1

# lhsT consts for the aug-build matmuls, [3, 20] f32:
#   cols 0-4 : A_t (targ coords -> rows 0..2)
#   cols 5-9 : B_t (targ squares -> row 4 = |b|^2)
#   cols 10-14: A_p (pred coords * -2 -> rows 0..2)
#   cols 15-19: B_p (pred squares -> row 3 = |a|^2)
_A_t = np.array([[1, 0, 0, 0, 0], [0, 1, 0, 0, 0], [0, 0, 1, 0, 0]], np.float32)
_B_t = np.array([[0, 0, 0, 0, 1], [0, 0, 0, 0, 1], [0, 0, 0, 0, 1]], np.float32)
_A_p = -2.0 * _A_t
_B_p = np.array([[0, 0, 0, 1, 0], [0, 0, 0, 1, 0], [0, 0, 0, 1, 0]], np.float32)
CONSTS = np.concatenate([_A_t, _B_t, _A_p, _B_p], axis=1)  # [3, 20]


def _build_aug(ctx, tc, aug, src_ap, cl_A, cl_B, ones_row, ones1, n):
    """Fill aug [5, n] bf16 from src [n, 3] DRAM coords.

    aug rows = (cl_A.T @ coords) accumulated with (cl_B.T @ coords^2); then
    row `ones_row` is set to 1.0 by DMA from the partition-0 ones tile
    (engine ops can't start at an unaligned partition; DMA can).
    """
    nc = tc.nc
    ng = n // G
    with tc.tile_pool(name="augb", bufs=1) as pool, \
         tc.tile_pool(name="augps", bufs=2, space="PSUM") as pspool:
        tb3 = pool.tile([3, n], F32)
        nc.sync.dma_start(tb3[:], src_ap.rearrange("j d -> d j"))
        with tc.tile_pool(name="augsq", bufs=2) as sqpool:
            for c in range(ng):
                sq = sqpool.tile([3, G], F32)
                nc.scalar.activation(sq[:], tb3[:, c * G:(c + 1) * G], AF.Square)
                ps = pspool.tile([5, G], F32)
                for k in range(G // FD):
                    j0 = c * G + k * FD
                    nc.tensor.matmul(ps[:, k * FD:(k + 1) * FD], cl_A,
                                     tb3[:, j0:j0 + FD], start=True, stop=False)
                    nc.tensor.matmul(ps[:, k * FD:(k + 1) * FD], cl_B,
                                     sq[:, k * FD:(k + 1) * FD], start=False,
                                     stop=True)
                nc.scalar.activation(aug[:, c * G:(c + 1) * G], ps[:], AF.Copy)
    for c in range(ng):
        nc.sync.dma_start(aug[ones_row:ones_row + 1, c * G:(c + 1) * G], ones1[:])


def diffusion_loss_kernel(ctx, tc, out_ap, ins, n=N):
    nc = tc.nc
    assert n % P == 0 and n % G == 0
    nt = n // P            # pred bands
    ng = n // G            # j groups
    npp = n // P
    wn = float(NOISE_WEIGHT / (B * n * 3))
    wc = float(CHAMFER_WEIGHT / (B * n))

    persist = ctx.enter_context(tc.tile_pool(name="persist", bufs=1))

    # ---------------- noise L1 loss ----------------
    noiseacc = persist.tile([P, 1], F32)
    with tc.tile_pool(name="noise", bufs=1) as nprep:
        pn_nat = nprep.tile([P, 3 * npp], F32)
        an_nat = nprep.tile([P, 3 * npp], F32)
        nc.sync.dma_start(pn_nat[:], ins["pn"].rearrange("(p f) d -> p (f d)", p=P))
        nc.sync.dma_start(an_nat[:], ins["an"].rearrange("(p f) d -> p (f d)", p=P))
        nc.vector.tensor_sub(pn_nat[:], pn_nat[:], an_nat[:])
        nc.vector.tensor_reduce(
            noiseacc[:], pn_nat[:], axis=AX.X, op=A.add, apply_absolute_value=True
        )

    # ---------------- aug operand build ----------------
    cl = persist.tile([3, 20], F32)
    nc.sync.dma_start(cl[:], ins["consts"])
    aaug = persist.tile([5, n], BF16)   # pred:  [-2a | |a|^2 | 1]
    baug = persist.tile([5, n], BF16)   # targ:  [b | 1 | |b|^2]
    ones1 = persist.tile([1, G], BF16)
    nc.vector.memset(ones1[:], 1.0)
    _build_aug(ctx, tc, baug, ins["targ"], cl[:, 0:5], cl[:, 5:10], 3, ones1, n)
    _build_aug(ctx, tc, aaug, ins["pred"], cl[:, 10:15], cl[:, 15:20], 4, ones1, n)

    # ---------------- main band loop ----------------
    # Hybrid domains, sized to balance ScalarE vs VectorE:
    #  - group 0 (j in [0,G)) stays LINEAR: DVE copies PSUM->bf16, reduces the
    #    band row-min into rmL, and min-folds colacc[:, 0:G].
    #  - groups 1..ng-1 are EXP: ACT emits exp(-K*D) with accum_out row sums
    #    (softmin) and DVE max-folds colacc (pairs of groups per op).
    # colacc exp region pre-set to TINY so the final Ln never sees 0; linear
    # region pre-set huge for the min fold.
    colacc = persist.tile([P, n], BF16)
    nc.vector.memset(colacc[:, 0:G], 3.0e38)
    nc.vector.memset(colacc[:, G:n], TINY)
    rmL = persist.tile([P, nt], F32)       # exact row mins over group-0 js
    rmEs = persist.tile([P, nt], F32)      # sum_j exp(-K D), exp groups
    accE = persist.tile([P, 2 * ng], F32)  # per-group ACT accums (dbuf parity)

    # exp-group pairing: (1,2),(3,4),... and a trailing single if odd count
    epairs = [(g, g + 1) for g in range(1, ng - 1, 2)]
    esingles = [ng - 1] if (ng - 1) % 2 == 1 else []

    # Col-folds and the accE gather are DEFERRED by one band: within each
    # band the DVE queue runs [linear copy+rowmin, then last band's folds],
    # so the copy releases group-0's PSUM slot immediately instead of
    # sitting behind ~9us of folds and starving PE/ACT.
    with tc.tile_pool(name="mainps", bufs=2, space="PSUM") as pspool, \
         tc.tile_pool(name="dtiles", bufs=2 * (len(epairs) + 1)) as dpool, \
         tc.tile_pool(name="dtiles1", bufs=2 * (len(esingles) + 1)) as d1pool:
        pending = []
        for t in range(nt):
            lhs = aaug[:, t * P:(t + 1) * P]
            po = (t % 2) * ng

            def mm_group(g, lhs=lhs):
                ps = pspool.tile([P, G], F32)
                for k in range(G // FD):
                    j0 = g * G + k * FD
                    nc.tensor.matmul(ps[:, k * FD:(k + 1) * FD], lhs,
                                     baug[:, j0:j0 + FD], start=True, stop=True)
                return ps

            # group 0: linear path on DVE (copy first -> frees PSUM slot)
            ps = mm_group(0)
            dtL = d1pool.tile([P, G], BF16, tag="dt1")
            nc.vector.tensor_copy(dtL[:], ps[:])

            # flush the previous band's deferred folds
            for f in pending:
                f()
            pending = []

            def rowredL(dtL=dtL, t=t):
                nc.vector.tensor_reduce(rmL[:, t:t + 1], dtL[:], axis=AX.X,
                                        op=A.min)
            pending.append(rowredL)

            def foldL(dtL=dtL):
                nc.vector.tensor_tensor(out=colacc[:, 0:G], in0=colacc[:, 0:G],
                                        in1=dtL[:], op=A.min)
            pending.append(foldL)

            # exp groups
            for ga, gb in epairs:
                dt = dpool.tile([P, 2 * G], BF16, tag="dt")
                for half, g in enumerate((ga, gb)):
                    ps = mm_group(g)
                    nc.scalar.activation(dt[:, half * G:(half + 1) * G], ps[:],
                                         AF.Exp, scale=-K_SOFT,
                                         accum_out=accE[:, po + g:po + g + 1])

                def foldE(dt=dt, gs=slice(ga * G, (gb + 1) * G)):
                    nc.vector.tensor_tensor(out=colacc[:, gs],
                                            in0=colacc[:, gs],
                                            in1=dt[:], op=A.max)
                pending.append(foldE)
            for g in esingles:
                dt1 = d1pool.tile([P, G], BF16, tag="dt1")
                ps = mm_group(g)
                nc.scalar.activation(dt1[:], ps[:], AF.Exp, scale=-K_SOFT,
                                     accum_out=accE[:, po + g:po + g + 1])

                def foldS(dt1=dt1, gs=slice(g * G, (g + 1) * G)):
                    nc.vector.tensor_tensor(out=colacc[:, gs],
                                            in0=colacc[:, gs],
                                            in1=dt1[:], op=A.max)
                pending.append(foldS)
            if ng > 1:
                def gatherE(t=t, po=po):
                    nc.vector.tensor_reduce(rmEs[:, t:t + 1],
                                            accE[:, po + 1:po + ng],
                                            axis=AX.X, op=A.add)
                pending.append(gatherE)
        for f in pending:
            f()

    # ---------------- epilogue ----------------
    with tc.tile_pool(name="epi", bufs=1) as epi:
        # rows: rm = min(rmL, -ln(sum exp)/K); empty exp-sums fall back to rmL
        if ng > 1:
            nc.vector.tensor_scalar_max(rmEs[:], rmEs[:], TINY)
            lnE = epi.tile([P, nt], F32)
            nc.scalar.activation(lnE[:], rmEs[:], AF.Ln)
            nc.vector.tensor_scalar_mul(lnE[:], lnE[:], -1.0 / K_SOFT)
            nc.vector.tensor_tensor(out=rmL[:], in0=rmL[:], in1=lnE[:],
                                    op=A.min)
        racc = epi.tile([P, 1], F32)
        nc.vector.tensor_reduce(racc[:], rmL[:], axis=AX.X, op=A.add)

        # cols: negate the linear slice, then one partition max-reduce
        nc.vector.tensor_scalar_mul(colacc[:, 0:G], colacc[:, 0:G], -1.0)
        allr = epi.tile([P, n], BF16)
        nc.gpsimd.partition_all_reduce(
            allr[:], colacc[:], channels=P, reduce_op=bass_isa.ReduceOp.max
        )
        # linear part: sum_j max(-D) = -colsum_linear
        csumL = epi.tile([1, 1], F32)
        nc.vector.tensor_reduce(csumL[:], allr[0:1, 0:G], axis=AX.X, op=A.add)
        # exp part: sum_j ln(max exp) = -K * colsum_exp
        if ng > 1:
            nc.scalar.activation(allr[0:1, G:n], allr[0:1, G:n], AF.Ln)
            csumE = epi.tile([1, 1], F32)
            nc.vector.tensor_reduce(csumE[:], allr[0:1, G:n], axis=AX.X,
                                    op=A.add)

        # per-partition v = noise*wn + rowsum*wc, then partition-sum
        v1 = epi.tile([P, 1], F32)
        nc.vector.tensor_scalar_mul(v1[:], noiseacc[:], wn)
        v = epi.tile([P, 1], F32)
        nc.vector.scalar_tensor_tensor(
            out=v[:], in0=racc[:], scalar=wc, in1=v1[:], op0=A.mult, op1=A.add
        )
        vv = epi.tile([P, 1], F32)
        nc.gpsimd.partition_all_reduce(
            vv[:], v[:], channels=P, reduce_op=bass_isa.ReduceOp.add
        )
        fs = epi.tile([1, 1], F32)
        nc.vector.scalar_tensor_tensor(
            out=fs[:], in0=csumL[:], scalar=-wc, in1=vv[0:1, :],
            op0=A.mult, op1=A.add,
        )
        if ng > 1:
            nc.vector.scalar_tensor_tensor(
                out=fs[:], in0=csumE[:], scalar=-wc / K_SOFT, in1=fs[:],
                op0=A.mult, op1=A.add,
            )
        nc.sync.dma_start(out_ap, fs[:])


_CACHE = {}


def build_program(n=N):
    key = (n,)
    if key not in _CACHE:
        nc = bacc.Bacc(
            "TRN2", target_bir_lowering=False, debug=False, enable_asserts=False
        )
        ins = {
            name: nc.dram_tensor(name, [n, 3], F32, kind="ExternalInput").ap()
            for name in ("pn", "an", "pred", "targ")
        }
        ins["consts"] = nc.dram_tensor(
            "consts", [3, 20], F32, kind="ExternalInput"
        ).ap()
        out_ap = nc.dram_tensor("out", [1, 1], F32, kind="ExternalOutput").ap()
        with tile.TileContext(nc) as tc:
            with ExitStack() as ctx:
                diffusion_loss_kernel(ctx, tc, out_ap, ins, n=n)
        nc.compile()
        _CACHE[key] = nc
    return _CACHE[key]


# ---------------------------------------------------------------------------
# Cached PJRT runner: build the jax.jit(shard_map) callable ONCE per Bass
# module and reuse it.  bass2jax.run_bass_via_pjrt builds a fresh closure per
# call, which re-traces/re-loads the program every run (~20-70us per static
# instruction through the axon tunnel); caching removes that entirely.
# ---------------------------------------------------------------------------
_RUNNER_CACHE = {}


def _get_runner(nc, n_cores=NCORES):
    key = (id(nc), n_cores)
    if key in _RUNNER_CACHE:
        return _RUNNER_CACHE[key]
    bass2jax.install_neuronx_cc_hook()
    partition_name = nc.partition_id_tensor.name if nc.partition_id_tensor else None
    in_names, out_names, out_avals, zero_shapes = [], [], [], []
    for alloc in nc.m.functions[0].allocations:
        if not isinstance(alloc, mybir.MemoryLocationSet):
            continue
        name = alloc.memorylocations[0].name
        if alloc.kind == "ExternalInput":
            if name != partition_name:
                in_names.append(name)
        elif alloc.kind == "ExternalOutput":
            shape = tuple(alloc.tensor_shape)
            dtype = mybir.dt.np(alloc.dtype)
            out_avals.append(jax.core.ShapedArray(shape, dtype))
            zero_shapes.append(((n_cores * shape[0], *shape[1:]), dtype))
            out_names.append(name)
    n_params = len(in_names)
    n_outs = len(out_avals)
    all_in = list(in_names) + list(out_names)
    if partition_name is not None:
        all_in.append(partition_name)
    donate = tuple(range(n_params, n_params + n_outs))

    def _body(*args):
        operands = list(args)
        if partition_name is not None:
            operands.append(bass2jax.partition_id_tensor())
        outs = bass2jax._bass_exec_p.bind(
            *operands,
            out_avals=tuple(out_avals),
            in_names=tuple(all_in),
            out_names=tuple(out_names),
            lowering_input_output_aliases=(),
            sim_require_finite=True,
            sim_require_nnan=True,
            nc=nc,
        )
        return tuple(outs)

    devices = jax.devices()[:n_cores]
    mesh = Mesh(np.asarray(devices), ("core",))
    in_specs = (PartitionSpec("core"),) * (n_params + n_outs)
    out_specs = (PartitionSpec("core"),) * n_outs
    sharded = jax.jit(
        shard_map(_body, mesh=mesh, in_specs=in_specs, out_specs=out_specs,
                  check_rep=False),
        donate_argnums=donate, keep_unused=True,
    )

    def run(concat_by_name):
        args = [concat_by_name[nm] for nm in in_names]
        zeros = [np.zeros(s, d) for s, d in zero_shapes]
        out_arrs = sharded(*args, *zeros)
        return {nm: np.asarray(out_arrs[i]) for i, nm in enumerate(out_names)}

    _RUNNER_CACHE[key] = run
    return run


def run_cores(inputs, n=N, trace=False):
    """Run the SPMD program over the batch; returns (partials, results)."""
    nc = build_program(n=n)
    pn = np.asarray(inputs["predicted_noise"], np.float32)
    an = np.asarray(inputs["actual_noise"], np.float32)
    pred = np.asarray(inputs["predicted_points_coarse"], np.float32)
    targ = np.asarray(inputs["target_points_coarse"], np.float32)
    nb = pn.shape[0]
    run = _get_runner(nc, n_cores=nb)
    outs = run({
        "pn": np.ascontiguousarray(pn).reshape(nb * n, 3),
        "an": np.ascontiguousarray(an).reshape(nb * n, 3),
        "pred": np.ascontiguousarray(pred).reshape(nb * n, 3),
        "targ": np.ascontiguousarray(targ).reshape(nb * n, 3),
        "consts": np.tile(CONSTS, (nb, 1)),
    })
    partials = outs["out"].reshape(nb)
    return partials, outs


def kernel(predicted_noise, actual_noise, predicted_points_coarse,
           target_points_coarse):
    partials, _ = run_cores(
        {
            "predicted_noise": predicted_noise,
            "actual_noise": actual_noise,
            "predicted_points_coarse": predicted_points_coarse,
            "target_points_coarse": target_points_coarse,
        }
    )
    return np.array(np.sum(partials, dtype=np.float32), dtype=np.float32)
